# revision 12
# baseline (speedup 1.0000x reference)
"""Trainium2 Bass kernel: Ernie4.5 VisionAttention (varlen attention, 4x512
segments, 16 heads x 80 dim, embed 1280).

Sharding: 8 cores = 2 segment-groups (2x512 tokens each) x 4 head-groups
(4 heads each). Tensor-parallel over heads (qkv column-shard, proj row-shard),
data-parallel over segment pairs. No collectives: per-core proj partials are
summed on the host.

v3: heads interleaved in the packed qk projection [q0 k0 q1 k1 ...] so head
j's rotated q/k depends only on packed tiles ~j (attention overlaps the
projection). Inputs arrive via 13 large DMAs (dma dispatch costs ~0.6us of
issuing-engine time each, so count matters more than size). Softmax
denominator accumulates into partition 96 of the ctx PSUM bank; 1/den is
broadcast across partitions with a K=1 PE matmul (tile_position=(96,0)).
Output in fp16 via 2 batched DMAs per segment.

Compute dtype: bf16 operands, fp32 PSUM accumulation.
"""

import sys

if "/opt/trn_rl_repo" not in sys.path:
    sys.path.insert(0, "/opt/trn_rl_repo")

import numpy as np
import ml_dtypes

BF = ml_dtypes.bfloat16

EMBED = 1280
HEADS = 16
HD = 80          # head dim
RH = 40          # rotary half
SEQ = 2048
SEGLEN = 512
N_CORES = 8
HPC = 4          # heads per core
TOK = 1024       # tokens per core (2 segments)
NSEG = 2
NUNITS = 2 * HPC # unit 2j = q of head j, unit 2j+1 = k of head j
VW = HD          # v block width per head
VTOT = HPC * VW  # 320
SCALE = HD ** -0.5
KCH = EMBED // 128  # 10

_CACHE = {}

# unpack pieces: packed row 80u+d lives in tile t=(80u+d)//128; piece list
# per packed tile t: (unit, unit_row_offset, tile_row_offset, length)
UNPACK_PIECES = {t: [] for t in range(5)}
for _u in range(NUNITS):
    _a = HD * _u
    while _a < HD * (_u + 1):
        _t = _a // 128
        _b = min(HD * (_u + 1), 128 * (_t + 1))
        UNPACK_PIECES[_t].append((_u, _a - HD * _u, _a - 128 * _t, _b - _a))
        _a = _b

# pi-swap source blocks per packed tile t (rows shift by +-40 inside each
# 80-row unit => sources span tiles t-1..t+1)
PI_BLOCKS = {0: [0, 1], 1: [0, 1, 2], 2: [1, 2, 3], 3: [2, 3, 4], 4: [3, 4]}


def _build_program():
    import concourse.tile as tile
    from concourse import bacc, mybir

    f32 = mybir.dt.float32
    f16 = mybir.dt.float16
    bf16 = mybir.dt.bfloat16
    AF = mybir.ActivationFunctionType
    ALU = mybir.AluOpType

    nc = bacc.Bacc("TRN2", target_bir_lowering=False, debug=False,
                   num_devices=N_CORES)

    xt_d = nc.dram_tensor("xt", [EMBED, TOK], bf16, kind="ExternalInput").ap()
    wqk_d = nc.dram_tensor("wqk", [EMBED, NUNITS * HD], bf16,
                           kind="ExternalInput").ap()
    wv_d = nc.dram_tensor("wv", [EMBED, VTOT], bf16, kind="ExternalInput").ap()
    vpat_d = nc.dram_tensor("vpat", [128, VTOT], bf16,
                            kind="ExternalInput").ap()
    wp_d = nc.dram_tensor("wp", [128, HPC * EMBED], bf16,
                          kind="ExternalInput").ap()
    bias_d = nc.dram_tensor("biasqk", [128, 5], f32, kind="ExternalInput").ap()
    cos_d = nc.dram_tensor("cosm", [NUNITS * HD, TOK], bf16,
                           kind="ExternalInput").ap()
    sin_d = nc.dram_tensor("sinm", [NUNITS * HD, TOK], bf16,
                           kind="ExternalInput").ap()
    pit_d = nc.dram_tensor("pit", [NUNITS * HD, NUNITS * HD], bf16,
                           kind="ExternalInput").ap()
    out_d = nc.dram_tensor("outT", [EMBED, TOK], f16, kind="ExternalOutput").ap()

    def load_grouped(dst_tile, src_ap, src_rows, col_w, groups):
        """DMA chunk groups of a [(C*128), col_w] DRAM tensor into a
        [128, C*col_w] SBUF tile; `groups` is a list of (e0, e1) chunk
        ranges, one dma_start each."""
        for e0, e1 in groups:
            n = e1 - e0
            src = src_ap[128 * e0:128 * e1, :].rearrange(
                "(e p) t -> p e t", p=128)
            dst = dst_tile[:, col_w * e0:col_w * e1].rearrange(
                "p (e t) -> p e t", t=col_w)
            nc.sync.dma_start(dst, src)

    with tile.TileContext(nc) as tc:
        with tc.tile_pool(name="persist", bufs=1) as P:
            # ---- batched persistent loads (13 dma dispatches) ----
            bias_sb = P.tile([128, 5], f32, name="biasqk_sb", tag="biasqk")
            nc.sync.dma_start(bias_sb[:], bias_d[:])
            xt_sb = P.tile([128, KCH * TOK], bf16, name="xt_sb", tag="xt")
            wqk_sb = P.tile([128, KCH * NUNITS * HD], bf16, name="wqk_sb",
                            tag="wqk")
            wv_sb = P.tile([128, KCH * VTOT], bf16, name="wv_sb", tag="wv")
            load_grouped(xt_sb, xt_d, EMBED, TOK, [(0, 2)])
            load_grouped(wqk_sb, wqk_d, EMBED, NUNITS * HD, [(0, 2)])
            load_grouped(wv_sb, wv_d, EMBED, VTOT, [(0, 2)])
            load_grouped(xt_sb, xt_d, EMBED, TOK, [(2, 6), (6, 10)])
            load_grouped(wqk_sb, wqk_d, EMBED, NUNITS * HD, [(2, 10)])
            load_grouped(wv_sb, wv_d, EMBED, VTOT, [(2, 10)])
            vpat_sb = P.tile([128, VTOT], bf16, name="vpat_sb", tag="vpat")
            nc.sync.dma_start(vpat_sb[:], vpat_d[:])
            cos_sb = P.tile([128, 5 * TOK], bf16, name="cos_sb", tag="cos")
            load_grouped(cos_sb, cos_d, NUNITS * HD, TOK, [(0, 5)])
            sin_sb = P.tile([128, 5 * TOK], bf16, name="sin_sb", tag="sin")
            load_grouped(sin_sb, sin_d, NUNITS * HD, TOK, [(0, 5)])
            pit_sb = P.tile([128, 5 * NUNITS * HD], bf16, name="pit_sb",
                            tag="pit")
            load_grouped(pit_sb, pit_d, NUNITS * HD, NUNITS * HD, [(0, 5)])
            # wp packed host-side as [128 rows(=head dim, 80 used),
            #  4 heads x 1280 embed]
            wp_sb = P.tile([128, HPC * EMBED], bf16, name="wp_sb", tag="wp")
            nc.sync.dma_start(wp_sb[:], wp_d[:])

            def xt(e):
                return xt_sb[:, TOK * e:TOK * (e + 1)]

            def wqk(e):
                return wqk_sb[:, NUNITS * HD * e:NUNITS * HD * (e + 1)]

            def wv(e):
                return wv_sb[:, VTOT * e:VTOT * (e + 1)]

            # persistent intermediates
            qkp_sb = [[None] * NSEG for _ in range(5)]
            qrot = [P.tile([HD, TOK], bf16, name=f"qrot{u}", tag=f"qrot{u}")
                    for u in range(NUNITS)]
            v_sb = [P.tile([128, VTOT], bf16, name=f"vsb{m}", tag=f"vsb{m}")
                    for m in range(TOK // 128)]
            ctxn = [[P.tile([HD, SEGLEN], bf16, name=f"ctxn{j}_{s}",
                            tag=f"ctxn{j}_{s}")
                     for s in range(NSEG)] for j in range(HPC)]
            o_sb = [P.tile([128, KCH * SEGLEN], f16, name=f"osb{s}",
                           tag=f"osb{s}") for s in range(NSEG)]

            # PSUM budget (8 banks): qk-proj, pi-swap, out-proj and the
            # 1/den broadcast share a 3-slot pool; v 1; scores 2; ctx 2.
            with tc.tile_pool(name="ps_a", bufs=3, space="PSUM") as PSA, \
                 tc.tile_pool(name="ps_v", bufs=1, space="PSUM") as PSV, \
                 tc.tile_pool(name="ps_st", bufs=2, space="PSUM") as PST, \
                 tc.tile_pool(name="ps_ctx", bufs=2, space="PSUM") as PSC, \
                 tc.tile_pool(name="work", bufs=3) as W, \
                 tc.tile_pool(name="workd", bufs=6) as WD:

                ones_sb = P.tile([128, 1], bf16, name="ones_sb", tag="ones")
                nc.vector.memset(ones_sb[:], 1.0)
                ones80 = P.tile([128, HD], f32, name="ones80", tag="ones80")
                nc.vector.memset(ones80[:], 1.0)

                est = {}    # (s, j) -> list of 4 exp'd score tiles

                def qkproj(t, s):
                    sc = slice(SEGLEN * s, SEGLEN * (s + 1))
                    qk_ps = PSA.tile([128, SEGLEN], f32, name=f"qkps{t}_{s}",
                                     tag="mm512")
                    for e in range(KCH):
                        nc.tensor.matmul(qk_ps[:],
                                         wqk(e)[:, 128 * t:128 * (t + 1)],
                                         xt(e)[:, sc],
                                         start=(e == 0), stop=(e == KCH - 1))
                    q_sb = W.tile([128, SEGLEN], bf16, name=f"qsb{t}_{s}",
                                  tag="qsb", bufs=10)
                    nc.scalar.activation(q_sb[:], qk_ps[:], AF.Identity,
                                         bias=bias_sb[:, t:t + 1])
                    qkp_sb[t][s] = q_sb

                def vchunk(m):
                    mc = slice(128 * m, 128 * (m + 1))
                    v_ps = PSV.tile([128, VTOT], f32, name=f"vps{m}", tag="vps")
                    for e in range(KCH):
                        nc.tensor.matmul(v_ps[:], xt(e)[:, mc], wv(e),
                                         start=(e == 0), stop=(e == KCH - 1))
                    # v_sb = v_ps + v-bias row pattern
                    nc.vector.scalar_tensor_tensor(v_sb[m][:], v_ps[:], 1.0,
                                                   vpat_sb[:], ALU.mult,
                                                   ALU.add)

                def rotary(tr, s):
                    sc = slice(SEGLEN * s, SEGLEN * (s + 1))
                    qsw_ps = PSA.tile([128, SEGLEN], f32, name=f"qsw{tr}_{s}",
                                      tag="mm512")
                    srcs = PI_BLOCKS[tr]
                    for i, tp in enumerate(srcs):
                        nc.tensor.matmul(qsw_ps[:],
                                         pit_sb[:, NUNITS * HD * tp + 128 * tr:
                                                NUNITS * HD * tp + 128 * (tr + 1)],
                                         qkp_sb[tp][s][:],
                                         start=(i == 0),
                                         stop=(i == len(srcs) - 1))
                    t1 = W.tile([128, SEGLEN], f32, name=f"t1_{tr}_{s}",
                                tag="t1", bufs=4)
                    nc.vector.tensor_tensor(t1[:], qkp_sb[tr][s][:],
                                            cos_sb[:, TOK * tr + sc.start:
                                                   TOK * tr + sc.stop],
                                            ALU.mult)
                    t2 = W.tile([128, SEGLEN], f32, name=f"t2_{tr}_{s}",
                                tag="t2", bufs=4)
                    nc.vector.tensor_tensor(t2[:], qsw_ps[:],
                                            sin_sb[:, TOK * tr + sc.start:
                                                   TOK * tr + sc.stop],
                                            ALU.mult)
                    rp = W.tile([128, SEGLEN], bf16, name=f"rotp{tr}_{s}",
                                tag="rotp", bufs=6)
                    nc.vector.tensor_tensor(rp[:], t1[:], t2[:], ALU.add)
                    for (u, po, toff, ln) in UNPACK_PIECES[tr]:
                        nc.sync.dma_start(qrot[u][po:po + ln, sc],
                                          rp[toff:toff + ln, :])

                def scores(s, j):
                    sc = slice(SEGLEN * s, SEGLEN * (s + 1))
                    lst = []
                    for tkc in range(SEGLEN // 128):
                        kc = slice(SEGLEN * s + 128 * tkc,
                                   SEGLEN * s + 128 * (tkc + 1))
                        st_ps = PST.tile([128, SEGLEN], f32,
                                         name=f"st{j}_{s}_{tkc}", tag="stps")
                        nc.tensor.matmul(st_ps[:], qrot[2 * j + 1][:, kc],
                                         qrot[2 * j][:, sc],
                                         start=True, stop=True)
                        e_sb = WD.tile([128, SEGLEN], bf16,
                                       name=f"est{j}_{s}_{tkc}", tag="est",
                                       bufs=8)
                        nc.scalar.activation(e_sb[:], st_ps[:], AF.Exp)
                        lst.append(e_sb)
                    est[(s, j)] = lst

                def ctx(s, j):
                    lst = est.pop((s, j))
                    ctx_ps = PSC.tile([128, SEGLEN], f32, name=f"ctxps{j}_{s}",
                                      tag="ctxps")
                    for tkc in range(SEGLEN // 128):
                        nc.tensor.matmul(ctx_ps[96:97, :], ones_sb[:],
                                         lst[tkc][:], start=(tkc == 0),
                                         stop=(tkc == 3),
                                         tile_position=(0, 96))
                        nc.tensor.matmul(ctx_ps[0:HD, :],
                                         v_sb[4 * s + tkc][:, VW * j:VW * (j + 1)],
                                         lst[tkc][:],
                                         start=(tkc == 0), stop=(tkc == 3))
                    # 1/den at partition 96, then K=1 matmul broadcast to 0..79
                    rec = WD.tile([128, SEGLEN], f32, name=f"rec{j}_{s}",
                                  tag="rec", bufs=2)
                    nc.vector.tensor_copy(rec[96:97, :], ctx_ps[96:97, :])
                    nc.vector.reciprocal_approx_fast(rec[:, :], rec[:, :])
                    bc_ps = PSA.tile([HD, SEGLEN], f32, name=f"bc{j}_{s}",
                                     tag="mm512")
                    nc.tensor.matmul(bc_ps[:], ones80[96:97, :], rec[96:97, :],
                                     start=True, stop=True,
                                     tile_position=(96, 0))
                    ctx_sb = WD.tile([HD, SEGLEN], f32, name=f"ctxsb{j}_{s}",
                                     tag="ctxsb", bufs=2)
                    nc.scalar.activation(ctx_sb[:], ctx_ps[0:HD, :],
                                         AF.Identity)
                    nc.vector.tensor_tensor(ctxn[j][s][:], ctx_sb[:],
                                            bc_ps[:], ALU.mult)

                def oproj(e, s):
                    o_ps = PSA.tile([128, SEGLEN], f32, name=f"ops{e}_{s}",
                                    tag="mm512")
                    for j in range(HPC):
                        nc.tensor.matmul(o_ps[:],
                                         wp_sb[0:HD, EMBED * j + 128 * e:
                                               EMBED * j + 128 * (e + 1)],
                                         ctxn[j][s][:],
                                         start=(j == 0), stop=(j == HPC - 1))
                    oc = o_sb[s][:, SEGLEN * e:SEGLEN * (e + 1)]
                    if e % 2 == 0:
                        nc.vector.tensor_copy(oc, o_ps[:])
                    else:
                        nc.scalar.activation(oc, o_ps[:], AF.Identity)

                def ostore(s, half):
                    e0, e1 = (0, 5) if half == 0 else (5, KCH)
                    src = o_sb[s][:, SEGLEN * e0:SEGLEN * e1].rearrange(
                        "p (e t) -> p e t", t=SEGLEN)
                    dst = out_d[128 * e0:128 * e1,
                                SEGLEN * s:SEGLEN * (s + 1)].rearrange(
                        "(e p) t -> p e t", p=128)
                    nc.sync.dma_start(dst, src)

                # ---- phase B: packed qk projection + v + rotary, pipelined;
                # attention for head j gates only on packed tiles <= j+1 ----
                VCH = {0: [0, 1, 2], 1: [3, 4, 5], 2: [6, 7], 3: [], 4: []}
                for t in range(5):
                    for s in range(NSEG):
                        qkproj(t, s)
                    for m in VCH[t]:
                        vchunk(m)
                    ready = [t - 1] if t < 4 else [3, 4]
                    for tr in ready:
                        if tr < 0:
                            continue
                        for s in range(NSEG):
                            rotary(tr, s)
                    if t >= 2:
                        j = t - 2
                        for s in range(NSEG):
                            scores(s, j)
                            ctx(s, j)

                # remaining head (j=3 needs rotary t4) -> interleave with
                # out-proj of seg 0 to keep PE dense
                scores(0, 3)
                ctx(0, 3)
                oproj(0, 0)
                oproj(1, 0)
                scores(1, 3)
                oproj(2, 0)
                oproj(3, 0)
                ctx(1, 3)
                oproj(4, 0)
                ostore(0, 0)
                for e in range(5, KCH):
                    oproj(e, 0)
                ostore(0, 1)
                for e in range(KCH):
                    oproj(e, 1)
                    if e == 4:
                        ostore(1, 0)
                ostore(1, 1)

    nc.compile()
    return nc


def _prep_inputs(x, rotary_pos_emb, qkv_w, qkv_b):
    """Build per-core input shards (host-side layout/constant prep)."""
    x2 = np.asarray(x, np.float32).reshape(SEQ, EMBED)
    rope = np.asarray(rotary_pos_emb, np.float32)
    qkv_w = np.asarray(qkv_w, np.float32)
    qkv_b = np.asarray(qkv_b, np.float32)

    # packed rotary multipliers: packed row p = 80u + d -> r = d % 40
    r_idx = np.tile(np.arange(HD) % RH, NUNITS)      # [640]
    cos_full = np.cos(rope)[:, r_idx].T.astype(BF)   # [640, 2048]
    sin_full = np.sin(rope)[:, r_idx].T.astype(BF)

    # packed swap permutation (sign folded), block-diagonal per 80-row unit:
    # within a unit, row d<40 reads -(d+40), row d>=40 reads +(d-40)
    D = NUNITS * HD
    Pi = np.zeros((D, D), np.float32)
    for u in range(NUNITS):
        o = HD * u
        for i in range(RH):
            Pi[o + i, o + i + RH] = -1.0
            Pi[o + i + RH, o + i] = 1.0
    pit = np.ascontiguousarray(Pi.T).astype(BF)

    in_maps = []
    for c in range(N_CORES):
        sg, hg = divmod(c, HPC)
        toks = slice(TOK * sg, TOK * (sg + 1))
        heads = [HPC * hg + j for j in range(HPC)]

        xa = np.ascontiguousarray(x2[toks].T)

        # interleaved packing: unit 2j = q of head j, unit 2j+1 = k
        wqk = np.empty((EMBED, NUNITS * HD), np.float32)
        bias_flat = np.empty(NUNITS * HD, np.float32)
        for j, h in enumerate(heads):
            oq, ok = HD * 2 * j, HD * (2 * j + 1)
            wqk[:, oq:oq + HD] = qkv_w[HD * h:HD * (h + 1), :].T * SCALE
            bias_flat[oq:oq + HD] = qkv_b[HD * h:HD * (h + 1)] * SCALE
            ko = EMBED + HD * h
            wqk[:, ok:ok + HD] = qkv_w[ko:ko + HD, :].T
            bias_flat[ok:ok + HD] = qkv_b[ko:ko + HD]
        bias = np.ascontiguousarray(bias_flat.reshape(5, 128).T)

        # v weights: per head block of 80 cols; eviction adds the v bias row
        wv = np.zeros((EMBED, VTOT), np.float32)
        vpat_row = np.zeros(VTOT, np.float32)
        for j, h in enumerate(heads):
            vo = 2 * EMBED + HD * h
            wv[:, VW * j:VW * j + HD] = qkv_w[vo:vo + HD, :].T
            vpat_row[VW * j:VW * j + HD] = qkv_b[vo:vo + HD]
        vpat = np.ascontiguousarray(np.broadcast_to(vpat_row, (128, VTOT)))

        # wp packed [128, 4*1280]: partition p (<80) of block j = row p of
        # wp_j ( = proj_w[:, 80h+p].T )
        wp = np.zeros((128, HPC * EMBED), np.float32)
        for j, h in enumerate(heads):
            wp[:HD, EMBED * j:EMBED * (j + 1)] = \
                _PROJ_W[:, HD * h:HD * (h + 1)].T
        wp = np.ascontiguousarray(wp)

        in_maps.append({
            "xt": xa.astype(BF),
            "wqk": wqk.astype(BF),
            "wv": wv.astype(BF),
            "vpat": vpat.astype(BF),
            "wp": wp.astype(BF),
            "biasqk": bias,
            "cosm": np.ascontiguousarray(cos_full[:, toks]),
            "sinm": np.ascontiguousarray(sin_full[:, toks]),
            "pit": pit,
        })
    return in_maps


_PROJ_W = None


def run_on_device(inputs, trace=False, trace_cores=None):
    """Shard, run on 8 NeuronCores, gather. Returns (output, BassKernelResults)."""
    global _PROJ_W
    from concourse import bass_utils

    x = np.asarray(inputs["x"], np.float32)
    cu = np.asarray(inputs["cu_seqlens"]).tolist()
    assert cu == [0, 512, 1024, 1536, 2048], (
        f"kernel compiled for 4x512 segments, got cu_seqlens={cu}")
    assert x.shape == (SEQ, 1, EMBED)

    _PROJ_W = np.asarray(inputs["proj_w"], np.float32)
    in_maps = _prep_inputs(x, inputs["rotary_pos_emb"],
                           inputs["qkv_w"], inputs["qkv_b"])

    if "nc" not in _CACHE:
        _CACHE["nc"] = _build_program()
    nc = _CACHE["nc"]

    kw = {}
    if trace:
        kw = dict(trace=True, trace_cores=trace_cores or [0])
    res = bass_utils.run_bass_kernel_spmd(nc, in_maps,
                                          core_ids=list(range(N_CORES)), **kw)

    proj_b = np.asarray(inputs["proj_b"], np.float32)
    out = np.empty((SEQ, EMBED), np.float32)
    for sg in range(2):
        acc = res.results[HPC * sg + 0]["outT"].astype(np.float32)
        for hg in range(1, HPC):
            acc = acc + res.results[HPC * sg + hg]["outT"].astype(np.float32)
        out[TOK * sg:TOK * (sg + 1)] = acc.T
    out += proj_b
    return out.reshape(SEQ, 1, EMBED), res


def kernel(**inputs):
    out, _ = run_on_device(inputs, trace=False)
    return out


# revision 13
# speedup vs baseline: 1.2621x; 1.2621x over previous
"""Trainium2 Bass kernel: Ernie4.5 VisionAttention (varlen attention, 4x512
segments, 16 heads x 80 dim, embed 1280).

Sharding: 8 cores = 2 segment-groups (2x512 tokens each) x 4 head-groups
(4 heads each). Tensor-parallel over heads (qkv column-shard, proj row-shard),
data-parallel over segment pairs. No collectives: per-core proj partials are
summed on the host.

v4: heads interleaved in the packed qk projection [q0 k0 q1 k1 ...] so head
j's rotated q/k depends only on packed tiles ~j (attention overlaps the
projection). x/wqk/wv are concatenated host-side into one "stream" tensor
loaded by five 2-chunk DMAs (dma dispatch costs ~0.6us of issuing-engine
time, and fine chunks keep the PE fed). The softmax denominator rides the
ctx matmul as a ones-column at stationary col 96 (partition 96 of the ctx
PSUM bank); 1/den is broadcast across partitions with a K=1 PE matmul
(tile_position=(96,0)). ctx rows are repacked via SBUF DMAs into three
K=128 tiles per segment so the output projection runs 3 full-K matmuls per
embed chunk. Output fp16 via 2 batched DMAs per segment.

Compute dtype: bf16 operands, fp32 PSUM accumulation.
"""

import sys

if "/opt/trn_rl_repo" not in sys.path:
    sys.path.insert(0, "/opt/trn_rl_repo")

import numpy as np
import ml_dtypes

BF = ml_dtypes.bfloat16

EMBED = 1280
HEADS = 16
HD = 80          # head dim
RH = 40          # rotary half
SEQ = 2048
SEGLEN = 512
N_CORES = 8
HPC = 4          # heads per core
TOK = 1024       # tokens per core (2 segments)
NSEG = 2
NUNITS = 2 * HPC # unit 2j = q of head j, unit 2j+1 = k of head j
VW = 97          # v block width per head (80 v + 16 pad + 1 ones col)
VTOT = HPC * VW  # 388
SW = TOK + NUNITS * HD + VTOT  # stream row: xt | wqk | wv = 1024+640+388
SCALE = HD ** -0.5
KCH = EMBED // 128  # 10

_CACHE = {}

# unpack pieces: packed row 80u+d lives in tile t=(80u+d)//128; piece list
# per packed tile t: (unit, unit_row_offset, tile_row_offset, length)
UNPACK_PIECES = {t: [] for t in range(5)}
for _u in range(NUNITS):
    _a = HD * _u
    while _a < HD * (_u + 1):
        _t = _a // 128
        _b = min(HD * (_u + 1), 128 * (_t + 1))
        UNPACK_PIECES[_t].append((_u, _a - HD * _u, _a - 128 * _t, _b - _a))
        _a = _b

# pi-swap source blocks per packed tile t (rows shift by +-40 inside each
# 80-row unit => sources span tiles t-1..t+1)
PI_BLOCKS = {0: [0, 1], 1: [0, 1, 2], 2: [1, 2, 3], 3: [2, 3, 4], 4: [3, 4]}

# ctx repack: head j rows 80j..80j+80 -> tile 80j//128 etc.
# (unit-row-offset, dst tile, dst row, length) pieces per head
CTXP_PIECES = {j: [] for j in range(HPC)}
for _j in range(HPC):
    _a = HD * _j
    while _a < HD * (_j + 1):
        _t = _a // 128
        _b = min(HD * (_j + 1), 128 * (_t + 1))
        CTXP_PIECES[_j].append((_a - HD * _j, _t, _a - 128 * _t, _b - _a))
        _a = _b


def _build_program():
    import concourse.tile as tile
    from concourse import bacc, mybir

    f32 = mybir.dt.float32
    f16 = mybir.dt.float16
    bf16 = mybir.dt.bfloat16
    AF = mybir.ActivationFunctionType
    ALU = mybir.AluOpType

    nc = bacc.Bacc("TRN2", target_bir_lowering=False, debug=False,
                   num_devices=N_CORES)

    stream_d = nc.dram_tensor("stream", [EMBED, SW], bf16,
                              kind="ExternalInput").ap()
    vpat_d = nc.dram_tensor("vpat", [128, VTOT], bf16,
                            kind="ExternalInput").ap()
    wp_d = nc.dram_tensor("wp", [128, 3 * EMBED], bf16,
                          kind="ExternalInput").ap()
    bias_d = nc.dram_tensor("biasqk", [128, 5], f32, kind="ExternalInput").ap()
    cos_d = nc.dram_tensor("cosm", [NUNITS * HD, TOK], bf16,
                           kind="ExternalInput").ap()
    sin_d = nc.dram_tensor("sinm", [NUNITS * HD, TOK], bf16,
                           kind="ExternalInput").ap()
    pit_d = nc.dram_tensor("pit", [NUNITS * HD, NUNITS * HD], bf16,
                           kind="ExternalInput").ap()
    out_d = nc.dram_tensor("outT", [EMBED, TOK], f16, kind="ExternalOutput").ap()

    def load_grouped(dst_tile, src_ap, col_w, groups):
        for e0, e1 in groups:
            src = src_ap[128 * e0:128 * e1, :].rearrange(
                "(e p) t -> p e t", p=128)
            dst = dst_tile[:, col_w * e0:col_w * e1].rearrange(
                "p (e t) -> p e t", t=col_w)
            nc.sync.dma_start(dst, src)

    with tile.TileContext(nc) as tc:
        with tc.tile_pool(name="persist", bufs=1) as P:
            # ---- batched persistent loads (14 dma dispatches), in
            # consumption order ----
            bias_sb = P.tile([128, 5], f32, name="biasqk_sb", tag="biasqk")
            nc.sync.dma_start(bias_sb[:], bias_d[:])
            vpat_sb = P.tile([128, VTOT], bf16, name="vpat_sb", tag="vpat")
            nc.sync.dma_start(vpat_sb[:], vpat_d[:])
            str_sb = P.tile([128, KCH * SW], bf16, name="str_sb", tag="str")
            cos_sb = P.tile([128, 5 * TOK], bf16, name="cos_sb", tag="cos")
            sin_sb = P.tile([128, 5 * TOK], bf16, name="sin_sb", tag="sin")
            pit_sb = P.tile([128, 5 * NUNITS * HD], bf16, name="pit_sb",
                            tag="pit")
            load_grouped(str_sb, stream_d, SW, [(0, 2), (2, 4)])
            load_grouped(cos_sb, cos_d, TOK, [(0, 2)])
            load_grouped(sin_sb, sin_d, TOK, [(0, 2)])
            load_grouped(pit_sb, pit_d, NUNITS * HD, [(0, 5)])
            load_grouped(str_sb, stream_d, SW, [(4, 6), (6, 8)])
            load_grouped(cos_sb, cos_d, TOK, [(2, 5)])
            load_grouped(sin_sb, sin_d, TOK, [(2, 5)])
            load_grouped(str_sb, stream_d, SW, [(8, 10)])
            wp_sb = P.tile([128, 3 * EMBED], bf16, name="wp_sb", tag="wp")
            nc.sync.dma_start(wp_sb[:], wp_d[:])

            def xt(e):
                return str_sb[:, SW * e:SW * e + TOK]

            def wqk(e):
                o = SW * e + TOK
                return str_sb[:, o:o + NUNITS * HD]

            def wv(e):
                o = SW * e + TOK + NUNITS * HD
                return str_sb[:, o:o + VTOT]

            # persistent intermediates
            qkp_sb = [[None] * NSEG for _ in range(5)]
            qrot = [P.tile([HD, TOK], bf16, name=f"qrot{u}", tag=f"qrot{u}")
                    for u in range(NUNITS)]
            v_sb = [P.tile([128, VTOT], bf16, name=f"vsb{m}", tag=f"vsb{m}")
                    for m in range(TOK // 128)]
            # repacked ctx: 3 tiles of 128 rows per segment (last 64 zero)
            ctxp = [[P.tile([128, SEGLEN], bf16, name=f"ctxp{c}_{s}",
                            tag=f"ctxp{c}_{s}") for c in range(3)]
                    for s in range(NSEG)]
            o_sb = [P.tile([128, KCH * SEGLEN], f16, name=f"osb{s}",
                           tag=f"osb{s}") for s in range(NSEG)]

            # PSUM budget (8 banks): qk-proj, pi-swap and out-proj share a
            # 3-slot pool; v 1; scores + 1/den broadcast 2; ctx 2.
            with tc.tile_pool(name="ps_a", bufs=3, space="PSUM") as PSA, \
                 tc.tile_pool(name="ps_v", bufs=1, space="PSUM") as PSV, \
                 tc.tile_pool(name="ps_st", bufs=2, space="PSUM") as PST, \
                 tc.tile_pool(name="ps_ctx", bufs=2, space="PSUM") as PSC, \
                 tc.tile_pool(name="work", bufs=3) as W, \
                 tc.tile_pool(name="workd", bufs=6) as WD:

                ones80 = P.tile([128, HD], f32, name="ones80", tag="ones80")
                nc.vector.memset(ones80[:], 1.0)
                # zero the 64 tail rows of the last ctx-repack tiles once
                for s in range(NSEG):
                    nc.vector.memset(ctxp[s][2][64:128, :], 0.0)

                est = {}    # (s, j) -> list of 4 exp'd score tiles

                def qkproj(t, s):
                    sc = slice(SEGLEN * s, SEGLEN * (s + 1))
                    qk_ps = PSA.tile([128, SEGLEN], f32, name=f"qkps{t}_{s}",
                                     tag="mm512")
                    for e in range(KCH):
                        nc.tensor.matmul(qk_ps[:],
                                         wqk(e)[:, 128 * t:128 * (t + 1)],
                                         xt(e)[:, sc],
                                         start=(e == 0), stop=(e == KCH - 1))
                    q_sb = W.tile([128, SEGLEN], bf16, name=f"qsb{t}_{s}",
                                  tag="qsb", bufs=10)
                    nc.scalar.activation(q_sb[:], qk_ps[:], AF.Identity,
                                         bias=bias_sb[:, t:t + 1])
                    qkp_sb[t][s] = q_sb

                def vchunk(m):
                    mc = slice(128 * m, 128 * (m + 1))
                    v_ps = PSV.tile([128, VTOT], f32, name=f"vps{m}", tag="vps")
                    for e in range(KCH):
                        nc.tensor.matmul(v_ps[:], xt(e)[:, mc], wv(e),
                                         start=(e == 0), stop=(e == KCH - 1))
                    # v_sb = v_ps + (v-bias | ones) row pattern
                    nc.vector.scalar_tensor_tensor(v_sb[m][:], v_ps[:], 1.0,
                                                   vpat_sb[:], ALU.mult,
                                                   ALU.add)

                def rotary(tr, s):
                    sc = slice(SEGLEN * s, SEGLEN * (s + 1))
                    qsw_ps = PSA.tile([128, SEGLEN], f32, name=f"qsw{tr}_{s}",
                                      tag="mm512")
                    srcs = PI_BLOCKS[tr]
                    for i, tp in enumerate(srcs):
                        nc.tensor.matmul(qsw_ps[:],
                                         pit_sb[:, NUNITS * HD * tp + 128 * tr:
                                                NUNITS * HD * tp + 128 * (tr + 1)],
                                         qkp_sb[tp][s][:],
                                         start=(i == 0),
                                         stop=(i == len(srcs) - 1))
                    t1 = W.tile([128, SEGLEN], bf16, name=f"t1_{tr}_{s}",
                                tag="t1", bufs=4)
                    nc.vector.tensor_tensor(t1[:], qkp_sb[tr][s][:],
                                            cos_sb[:, TOK * tr + sc.start:
                                                   TOK * tr + sc.stop],
                                            ALU.mult)
                    t2 = W.tile([128, SEGLEN], bf16, name=f"t2_{tr}_{s}",
                                tag="t2", bufs=4)
                    nc.vector.tensor_tensor(t2[:], qsw_ps[:],
                                            sin_sb[:, TOK * tr + sc.start:
                                                   TOK * tr + sc.stop],
                                            ALU.mult)
                    rp = W.tile([128, SEGLEN], bf16, name=f"rotp{tr}_{s}",
                                tag="rotp", bufs=6)
                    nc.vector.tensor_tensor(rp[:], t1[:], t2[:], ALU.add)
                    for (u, po, toff, ln) in UNPACK_PIECES[tr]:
                        nc.sync.dma_start(qrot[u][po:po + ln, sc],
                                          rp[toff:toff + ln, :])

                def scores(s, j):
                    sc = slice(SEGLEN * s, SEGLEN * (s + 1))
                    lst = []
                    for tkc in range(SEGLEN // 128):
                        kc = slice(SEGLEN * s + 128 * tkc,
                                   SEGLEN * s + 128 * (tkc + 1))
                        st_ps = PST.tile([128, SEGLEN], f32,
                                         name=f"st{j}_{s}_{tkc}", tag="stps")
                        nc.tensor.matmul(st_ps[:], qrot[2 * j + 1][:, kc],
                                         qrot[2 * j][:, sc],
                                         start=True, stop=True)
                        e_sb = WD.tile([128, SEGLEN], bf16,
                                       name=f"est{j}_{s}_{tkc}", tag="est",
                                       bufs=10)
                        nc.scalar.activation(e_sb[:], st_ps[:], AF.Exp)
                        lst.append(e_sb)
                    est[(s, j)] = lst

                def ctx(s, j):
                    lst = est.pop((s, j))
                    ctx_ps = PSC.tile([128, SEGLEN], f32, name=f"ctxps{j}_{s}",
                                      tag="ctxps")
                    for tkc in range(SEGLEN // 128):
                        nc.tensor.matmul(ctx_ps[0:VW, :],
                                         v_sb[4 * s + tkc][:, VW * j:VW * (j + 1)],
                                         lst[tkc][:],
                                         start=(tkc == 0), stop=(tkc == 3))
                    # den accumulated into partition 96 (ones column of the
                    # v block); 1/den broadcast to 0..79 via K=1 matmul
                    rec = WD.tile([128, SEGLEN], f32, name=f"rec{j}_{s}",
                                  tag="rec", bufs=2)
                    nc.vector.tensor_copy(rec[96:97, :], ctx_ps[96:97, :])
                    nc.vector.reciprocal_approx_fast(rec[:, :], rec[:, :])
                    bc_ps = PST.tile([HD, SEGLEN], f32, name=f"bc{j}_{s}",
                                     tag="stps")
                    nc.tensor.matmul(bc_ps[:], ones80[96:97, :], rec[96:97, :],
                                     start=True, stop=True,
                                     tile_position=(96, 0))
                    ctx_sb = WD.tile([HD, SEGLEN], f32, name=f"ctxsb{j}_{s}",
                                     tag="ctxsb", bufs=2)
                    nc.scalar.activation(ctx_sb[:], ctx_ps[0:HD, :],
                                         AF.Identity)
                    ctxn = WD.tile([HD, SEGLEN], bf16, name=f"ctxn{j}_{s}",
                                   tag="ctxn", bufs=3)
                    nc.vector.tensor_tensor(ctxn[:], ctx_sb[:], bc_ps[:],
                                            ALU.mult)
                    # repack into the 3 K=128 out-proj tiles
                    for (po, c, toff, ln) in CTXP_PIECES[j]:
                        nc.sync.dma_start(ctxp[s][c][toff:toff + ln, :],
                                          ctxn[po:po + ln, :])

                def oproj(e, s):
                    o_ps = PSA.tile([128, SEGLEN], f32, name=f"ops{e}_{s}",
                                    tag="mm512")
                    for c in range(3):
                        nc.tensor.matmul(o_ps[:],
                                         wp_sb[:, EMBED * c + 128 * e:
                                               EMBED * c + 128 * (e + 1)],
                                         ctxp[s][c][:],
                                         start=(c == 0), stop=(c == 2))
                    oc = o_sb[s][:, SEGLEN * e:SEGLEN * (e + 1)]
                    if e % 2 == 0:
                        nc.vector.tensor_copy(oc, o_ps[:])
                    else:
                        nc.scalar.activation(oc, o_ps[:], AF.Identity)

                def ostore(s, half):
                    e0, e1 = (0, 5) if half == 0 else (5, KCH)
                    src = o_sb[s][:, SEGLEN * e0:SEGLEN * e1].rearrange(
                        "p (e t) -> p e t", t=SEGLEN)
                    dst = out_d[128 * e0:128 * e1,
                                SEGLEN * s:SEGLEN * (s + 1)].rearrange(
                        "(e p) t -> p e t", p=128)
                    nc.sync.dma_start(dst, src)

                # ---- phase B: packed qk projection + v + rotary, pipelined;
                # attention for head j gates only on packed tiles <= j+1 ----
                VCH = {0: [0, 1, 2], 1: [3, 4, 5], 2: [6, 7], 3: [], 4: []}
                for t in range(5):
                    for s in range(NSEG):
                        qkproj(t, s)
                    for m in VCH[t]:
                        vchunk(m)
                    ready = [t - 1] if t < 4 else [3, 4]
                    for tr in ready:
                        if tr < 0:
                            continue
                        for s in range(NSEG):
                            rotary(tr, s)
                    if t >= 2:
                        j = t - 2
                        for s in range(NSEG):
                            scores(s, j)
                            ctx(s, j)

                # remaining head (j=3 needs rotary t4) -> interleave with
                # out-proj of seg 0 to keep PE dense
                scores(0, 3)
                ctx(0, 3)
                oproj(0, 0)
                oproj(1, 0)
                scores(1, 3)
                oproj(2, 0)
                oproj(3, 0)
                ctx(1, 3)
                oproj(4, 0)
                ostore(0, 0)
                for e in range(5, KCH):
                    oproj(e, 0)
                ostore(0, 1)
                for e in range(KCH):
                    oproj(e, 1)
                    if e == 4:
                        ostore(1, 0)
                ostore(1, 1)

    nc.compile()
    return nc


def _prep_inputs(x, rotary_pos_emb, qkv_w, qkv_b):
    """Build per-core input shards (host-side layout/constant prep)."""
    x2 = np.asarray(x, np.float32).reshape(SEQ, EMBED)
    rope = np.asarray(rotary_pos_emb, np.float32)
    qkv_w = np.asarray(qkv_w, np.float32)
    qkv_b = np.asarray(qkv_b, np.float32)

    # packed rotary multipliers: packed row p = 80u + d -> r = d % 40
    r_idx = np.tile(np.arange(HD) % RH, NUNITS)      # [640]
    cos_full = np.cos(rope)[:, r_idx].T.astype(BF)   # [640, 2048]
    sin_full = np.sin(rope)[:, r_idx].T.astype(BF)

    # packed swap permutation (sign folded), block-diagonal per 80-row unit:
    # within a unit, row d<40 reads -(d+40), row d>=40 reads +(d-40)
    D = NUNITS * HD
    Pi = np.zeros((D, D), np.float32)
    for u in range(NUNITS):
        o = HD * u
        for i in range(RH):
            Pi[o + i, o + i + RH] = -1.0
            Pi[o + i + RH, o + i] = 1.0
    pit = np.ascontiguousarray(Pi.T).astype(BF)

    in_maps = []
    for c in range(N_CORES):
        sg, hg = divmod(c, HPC)
        toks = slice(TOK * sg, TOK * (sg + 1))
        heads = [HPC * hg + j for j in range(HPC)]

        xa = x2[toks].T                                   # [1280, 1024]

        # interleaved packing: unit 2j = q of head j, unit 2j+1 = k
        wqk = np.empty((EMBED, NUNITS * HD), np.float32)
        bias_flat = np.empty(NUNITS * HD, np.float32)
        for j, h in enumerate(heads):
            oq, ok = HD * 2 * j, HD * (2 * j + 1)
            wqk[:, oq:oq + HD] = qkv_w[HD * h:HD * (h + 1), :].T * SCALE
            bias_flat[oq:oq + HD] = qkv_b[HD * h:HD * (h + 1)] * SCALE
            ko = EMBED + HD * h
            wqk[:, ok:ok + HD] = qkv_w[ko:ko + HD, :].T
            bias_flat[ok:ok + HD] = qkv_b[ko:ko + HD]
        bias = np.ascontiguousarray(bias_flat.reshape(5, 128).T)

        # v weights: 97-wide blocks per head (80 v | 16 zero | 1 zero);
        # eviction adds vpat = (v-bias | 0 | 1.0) so the ctx matmul
        # accumulates the softmax denominator into partition 96
        wv = np.zeros((EMBED, VTOT), np.float32)
        vpat_row = np.zeros(VTOT, np.float32)
        for j, h in enumerate(heads):
            vo = 2 * EMBED + HD * h
            wv[:, VW * j:VW * j + HD] = qkv_w[vo:vo + HD, :].T
            vpat_row[VW * j:VW * j + HD] = qkv_b[vo:vo + HD]
            vpat_row[VW * j + 96] = 1.0
        vpat = np.ascontiguousarray(np.broadcast_to(vpat_row, (128, VTOT)))

        stream = np.concatenate([xa, wqk, wv], axis=1)    # [1280, SW]

        # wp packed for K=128 repacked ctx: rows = stacked head-dims
        # (320 real + 64 zero), 3 chunks of 128 side by side
        wp_cat = np.zeros((384, EMBED), np.float32)
        for j, h in enumerate(heads):
            wp_cat[HD * j:HD * (j + 1), :] = _PROJ_W[:, HD * h:HD * (h + 1)].T
        wp = np.zeros((128, 3 * EMBED), np.float32)
        for c_ in range(3):
            wp[:, EMBED * c_:EMBED * (c_ + 1)] = wp_cat[128 * c_:128 * (c_ + 1)]

        in_maps.append({
            "stream": np.ascontiguousarray(stream).astype(BF),
            "vpat": vpat.astype(BF),
            "wp": np.ascontiguousarray(wp).astype(BF),
            "biasqk": bias,
            "cosm": np.ascontiguousarray(cos_full[:, toks]),
            "sinm": np.ascontiguousarray(sin_full[:, toks]),
            "pit": pit,
        })
    return in_maps


_PROJ_W = None


def run_on_device(inputs, trace=False, trace_cores=None):
    """Shard, run on 8 NeuronCores, gather. Returns (output, BassKernelResults)."""
    global _PROJ_W
    from concourse import bass_utils

    x = np.asarray(inputs["x"], np.float32)
    cu = np.asarray(inputs["cu_seqlens"]).tolist()
    assert cu == [0, 512, 1024, 1536, 2048], (
        f"kernel compiled for 4x512 segments, got cu_seqlens={cu}")
    assert x.shape == (SEQ, 1, EMBED)

    _PROJ_W = np.asarray(inputs["proj_w"], np.float32)
    in_maps = _prep_inputs(x, inputs["rotary_pos_emb"],
                           inputs["qkv_w"], inputs["qkv_b"])

    if "nc" not in _CACHE:
        _CACHE["nc"] = _build_program()
    nc = _CACHE["nc"]

    kw = {}
    if trace:
        kw = dict(trace=True, trace_cores=trace_cores or [0])
    res = bass_utils.run_bass_kernel_spmd(nc, in_maps,
                                          core_ids=list(range(N_CORES)), **kw)

    proj_b = np.asarray(inputs["proj_b"], np.float32)
    out = np.empty((SEQ, EMBED), np.float32)
    for sg in range(2):
        acc = res.results[HPC * sg + 0]["outT"].astype(np.float32)
        for hg in range(1, HPC):
            acc = acc + res.results[HPC * sg + hg]["outT"].astype(np.float32)
        out[TOK * sg:TOK * (sg + 1)] = acc.T
    out += proj_b
    return out.reshape(SEQ, 1, EMBED), res


def kernel(**inputs):
    out, _ = run_on_device(inputs, trace=False)
    return out


# revision 18
# speedup vs baseline: 1.3941x; 1.1046x over previous
"""Trainium2 Bass kernel: Ernie4.5 VisionAttention (varlen attention, 4x512
segments, 16 heads x 80 dim, embed 1280).

Sharding: 8 cores = 2 segment-groups (2x512 tokens each) x 4 head-groups
(4 heads each). Tensor-parallel over heads (qkv column-shard, proj row-shard),
data-parallel over segment pairs. No collectives: per-core proj partials are
summed on the host.

v4: heads interleaved in the packed qk projection [q0 k0 q1 k1 ...] so head
j's rotated q/k depends only on packed tiles ~j (attention overlaps the
projection). x/wqk/wv are concatenated host-side into one "stream" tensor
loaded by five 2-chunk DMAs (dma dispatch costs ~0.6us of issuing-engine
time, and fine chunks keep the PE fed). The softmax denominator rides the
ctx matmul as a ones-column at stationary col 96 (partition 96 of the ctx
PSUM bank); 1/den is broadcast across partitions with a K=1 PE matmul
(tile_position=(96,0)). ctx rows are repacked via SBUF DMAs into three
K=128 tiles per segment so the output projection runs 3 full-K matmuls per
embed chunk. Output fp16 via 2 batched DMAs per segment.

Compute dtype: bf16 operands, fp32 PSUM accumulation.
"""

import sys

if "/opt/trn_rl_repo" not in sys.path:
    sys.path.insert(0, "/opt/trn_rl_repo")

import numpy as np
import ml_dtypes

BF = ml_dtypes.bfloat16

EMBED = 1280
HEADS = 16
HD = 80          # head dim
RH = 40          # rotary half
SEQ = 2048
SEGLEN = 512
N_CORES = 8
HPC = 4          # heads per core
TOK = 1024       # tokens per core (2 segments)
NSEG = 2
NUNITS = 2 * HPC # unit 2j = q of head j, unit 2j+1 = k of head j
VW = 97          # v block width per head (80 v + 16 pad + 1 ones col)
VTOT = HPC * VW  # 388
SW = TOK + NUNITS * HD + VTOT  # stream row: xt | wqk | wv = 1024+640+388
SCALE = HD ** -0.5
KCH = EMBED // 128  # 10

_CACHE = {}

# unpack pieces: packed row 80u+d lives in tile t=(80u+d)//128; piece list
# per packed tile t: (unit, unit_row_offset, tile_row_offset, length)
UNPACK_PIECES = {t: [] for t in range(5)}
for _u in range(NUNITS):
    _a = HD * _u
    while _a < HD * (_u + 1):
        _t = _a // 128
        _b = min(HD * (_u + 1), 128 * (_t + 1))
        UNPACK_PIECES[_t].append((_u, _a - HD * _u, _a - 128 * _t, _b - _a))
        _a = _b

# pi-swap source blocks per packed tile t (rows shift by +-40 inside each
# 80-row unit => sources span tiles t-1..t+1)
PI_BLOCKS = {0: [0, 1], 1: [0, 1, 2], 2: [1, 2, 3], 3: [2, 3, 4], 4: [3, 4]}

# ctx repack: head j rows 80j..80j+80 -> tile 80j//128 etc.
# (unit-row-offset, dst tile, dst row, length) pieces per head
CTXP_PIECES = {j: [] for j in range(HPC)}
for _j in range(HPC):
    _a = HD * _j
    while _a < HD * (_j + 1):
        _t = _a // 128
        _b = min(HD * (_j + 1), 128 * (_t + 1))
        CTXP_PIECES[_j].append((_a - HD * _j, _t, _a - 128 * _t, _b - _a))
        _a = _b


def _build_program():
    import concourse.tile as tile
    from concourse import bacc, mybir

    f32 = mybir.dt.float32
    f16 = mybir.dt.float16
    bf16 = mybir.dt.bfloat16
    AF = mybir.ActivationFunctionType
    ALU = mybir.AluOpType

    nc = bacc.Bacc("TRN2", target_bir_lowering=False, debug=False,
                   num_devices=N_CORES)

    stream_d = nc.dram_tensor("stream", [EMBED, SW], bf16,
                              kind="ExternalInput").ap()
    vpat_d = nc.dram_tensor("vpat", [128, VTOT], bf16,
                            kind="ExternalInput").ap()
    wp_d = nc.dram_tensor("wp", [128, 3 * EMBED], bf16,
                          kind="ExternalInput").ap()
    bias_d = nc.dram_tensor("biasqk", [128, 5], f32, kind="ExternalInput").ap()
    cos_d = nc.dram_tensor("cosm", [NUNITS * HD, TOK], bf16,
                           kind="ExternalInput").ap()
    sin_d = nc.dram_tensor("sinm", [NUNITS * HD, TOK], bf16,
                           kind="ExternalInput").ap()
    pit_d = nc.dram_tensor("pit", [NUNITS * HD, NUNITS * HD], bf16,
                           kind="ExternalInput").ap()
    out_d = nc.dram_tensor("outT", [EMBED, TOK], f16, kind="ExternalOutput").ap()

    def load_grouped(dst_tile, src_ap, col_w, groups):
        for e0, e1 in groups:
            src = src_ap[128 * e0:128 * e1, :].rearrange(
                "(e p) t -> p e t", p=128)
            dst = dst_tile[:, col_w * e0:col_w * e1].rearrange(
                "p (e t) -> p e t", t=col_w)
            nc.sync.dma_start(dst, src)

    with tile.TileContext(nc) as tc:
        with tc.tile_pool(name="persist", bufs=1) as P:
            # ---- batched persistent loads (14 dma dispatches), in
            # consumption order ----
            bias_sb = P.tile([128, 5], f32, name="biasqk_sb", tag="biasqk")
            nc.sync.dma_start(bias_sb[:], bias_d[:])
            vpat_sb = P.tile([128, VTOT], bf16, name="vpat_sb", tag="vpat")
            nc.sync.dma_start(vpat_sb[:], vpat_d[:])
            str_sb = P.tile([128, KCH * SW], bf16, name="str_sb", tag="str")
            cos_sb = P.tile([128, 5 * TOK], bf16, name="cos_sb", tag="cos")
            sin_sb = P.tile([128, 5 * TOK], bf16, name="sin_sb", tag="sin")
            pit_sb = P.tile([128, 5 * NUNITS * HD], bf16, name="pit_sb",
                            tag="pit")
            load_grouped(str_sb, stream_d, SW,
                         [(0, 2), (2, 4), (4, 6), (6, 8), (8, 10)])
            load_grouped(pit_sb, pit_d, NUNITS * HD, [(0, 2)])
            load_grouped(cos_sb, cos_d, TOK, [(0, 2)])
            load_grouped(sin_sb, sin_d, TOK, [(0, 2)])
            load_grouped(pit_sb, pit_d, NUNITS * HD, [(2, 5)])
            load_grouped(cos_sb, cos_d, TOK, [(2, 5)])
            load_grouped(sin_sb, sin_d, TOK, [(2, 5)])
            wp_sb = P.tile([128, 3 * EMBED], bf16, name="wp_sb", tag="wp")
            nc.sync.dma_start(wp_sb[:], wp_d[:])

            def xt(e):
                return str_sb[:, SW * e:SW * e + TOK]

            def wqk(e):
                o = SW * e + TOK
                return str_sb[:, o:o + NUNITS * HD]

            def wv(e):
                o = SW * e + TOK + NUNITS * HD
                return str_sb[:, o:o + VTOT]

            # persistent intermediates
            qkp_sb = [[None] * NSEG for _ in range(5)]
            qrot = [P.tile([HD, TOK], bf16, name=f"qrot{u}", tag=f"qrot{u}")
                    for u in range(NUNITS)]
            v_sb = [P.tile([128, VTOT], bf16, name=f"vsb{m}", tag=f"vsb{m}")
                    for m in range(TOK // 128)]
            # repacked ctx: 3 tiles of 128 rows per segment (last 64 zero)
            ctxp = [[P.tile([128, SEGLEN], bf16, name=f"ctxp{c}_{s}",
                            tag=f"ctxp{c}_{s}") for c in range(3)]
                    for s in range(NSEG)]
            o_sb = [P.tile([128, KCH * SEGLEN], f16, name=f"osb{s}",
                           tag=f"osb{s}") for s in range(NSEG)]

            # PSUM budget (8 banks): qk-proj, pi-swap and out-proj share a
            # 3-slot pool; v 1; scores + 1/den broadcast 2; ctx 2.
            with tc.tile_pool(name="ps_a", bufs=3, space="PSUM") as PSA, \
                 tc.tile_pool(name="ps_v", bufs=1, space="PSUM") as PSV, \
                 tc.tile_pool(name="ps_st", bufs=2, space="PSUM") as PST, \
                 tc.tile_pool(name="ps_ctx", bufs=2, space="PSUM") as PSC, \
                 tc.tile_pool(name="work", bufs=3) as W, \
                 tc.tile_pool(name="workd", bufs=6) as WD:

                ones80 = P.tile([128, HD], f32, name="ones80", tag="ones80")
                nc.vector.memset(ones80[:], 1.0)
                # zero the 64 tail rows of the last ctx-repack tiles once
                for s in range(NSEG):
                    nc.vector.memset(ctxp[s][2][64:128, :], 0.0)

                est = {}    # (s, j) -> list of 4 exp'd score tiles

                def qkproj(t, s):
                    sc = slice(SEGLEN * s, SEGLEN * (s + 1))
                    qk_ps = PSA.tile([128, SEGLEN], f32, name=f"qkps{t}_{s}",
                                     tag="mm512")
                    for e in range(KCH):
                        nc.tensor.matmul(qk_ps[:],
                                         wqk(e)[:, 128 * t:128 * (t + 1)],
                                         xt(e)[:, sc],
                                         start=(e == 0), stop=(e == KCH - 1))
                    q_sb = W.tile([128, SEGLEN], bf16, name=f"qsb{t}_{s}",
                                  tag="qsb", bufs=10)
                    nc.scalar.activation(q_sb[:], qk_ps[:], AF.Identity,
                                         bias=bias_sb[:, t:t + 1])
                    qkp_sb[t][s] = q_sb

                def vchunk(m):
                    mc = slice(128 * m, 128 * (m + 1))
                    v_ps = PSV.tile([128, VTOT], f32, name=f"vps{m}", tag="vps")
                    for e in range(KCH):
                        nc.tensor.matmul(v_ps[:], xt(e)[:, mc], wv(e),
                                         start=(e == 0), stop=(e == KCH - 1))
                    # v_sb = v_ps + (v-bias | ones) row pattern
                    nc.vector.scalar_tensor_tensor(v_sb[m][:], v_ps[:], 1.0,
                                                   vpat_sb[:], ALU.mult,
                                                   ALU.add)

                def rotary(tr, s):
                    sc = slice(SEGLEN * s, SEGLEN * (s + 1))
                    qsw_ps = PSA.tile([128, SEGLEN], f32, name=f"qsw{tr}_{s}",
                                      tag="mm512")
                    srcs = PI_BLOCKS[tr]
                    for i, tp in enumerate(srcs):
                        nc.tensor.matmul(qsw_ps[:],
                                         pit_sb[:, NUNITS * HD * tp + 128 * tr:
                                                NUNITS * HD * tp + 128 * (tr + 1)],
                                         qkp_sb[tp][s][:],
                                         start=(i == 0),
                                         stop=(i == len(srcs) - 1))
                    t1 = W.tile([128, SEGLEN], bf16, name=f"t1_{tr}_{s}",
                                tag="t1", bufs=4)
                    nc.vector.tensor_tensor(t1[:], qkp_sb[tr][s][:],
                                            cos_sb[:, TOK * tr + sc.start:
                                                   TOK * tr + sc.stop],
                                            ALU.mult)
                    t2 = W.tile([128, SEGLEN], bf16, name=f"t2_{tr}_{s}",
                                tag="t2", bufs=4)
                    nc.vector.tensor_tensor(t2[:], qsw_ps[:],
                                            sin_sb[:, TOK * tr + sc.start:
                                                   TOK * tr + sc.stop],
                                            ALU.mult)
                    rp = W.tile([128, SEGLEN], bf16, name=f"rotp{tr}_{s}",
                                tag="rotp", bufs=6)
                    nc.vector.tensor_tensor(rp[:], t1[:], t2[:], ALU.add)
                    for (u, po, toff, ln) in UNPACK_PIECES[tr]:
                        nc.sync.dma_start(qrot[u][po:po + ln, sc],
                                          rp[toff:toff + ln, :])

                def scores(s, j):
                    sc = slice(SEGLEN * s, SEGLEN * (s + 1))
                    lst = []
                    for tkc in range(SEGLEN // 128):
                        kc = slice(SEGLEN * s + 128 * tkc,
                                   SEGLEN * s + 128 * (tkc + 1))
                        st_ps = PST.tile([128, SEGLEN], f32,
                                         name=f"st{j}_{s}_{tkc}", tag="stps")
                        nc.tensor.matmul(st_ps[:], qrot[2 * j + 1][:, kc],
                                         qrot[2 * j][:, sc],
                                         start=True, stop=True)
                        e_sb = WD.tile([128, SEGLEN], bf16,
                                       name=f"est{j}_{s}_{tkc}", tag="est",
                                       bufs=14)
                        nc.scalar.activation(e_sb[:], st_ps[:], AF.Exp)
                        lst.append(e_sb)
                    est[(s, j)] = lst

                def ctx(s, j):
                    lst = est.pop((s, j))
                    ctx_ps = PSC.tile([128, SEGLEN], f32, name=f"ctxps{j}_{s}",
                                      tag="ctxps")
                    for tkc in range(SEGLEN // 128):
                        nc.tensor.matmul(ctx_ps[0:VW, :],
                                         v_sb[4 * s + tkc][:, VW * j:VW * (j + 1)],
                                         lst[tkc][:],
                                         start=(tkc == 0), stop=(tkc == 3))
                    # den accumulated into partition 96 (ones column of the
                    # v block); 1/den broadcast to 0..79 via K=1 matmul
                    rec = WD.tile([128, SEGLEN], f32, name=f"rec{j}_{s}",
                                  tag="rec", bufs=2)
                    nc.vector.reciprocal_approx_fast(rec[:, :], ctx_ps[:, :])
                    bc_ps = PSV.tile([HD, SEGLEN], f32, name=f"bc{j}_{s}",
                                     tag="vps")
                    nc.tensor.matmul(bc_ps[:], ones80[96:97, :], rec[96:97, :],
                                     start=True, stop=True,
                                     tile_position=(96, 0))
                    ctx_sb = WD.tile([HD, SEGLEN], f32, name=f"ctxsb{j}_{s}",
                                     tag="ctxsb", bufs=2)
                    if j % 2 == 0:
                        nc.scalar.activation(ctx_sb[:], ctx_ps[0:HD, :],
                                             AF.Identity)
                    else:
                        nc.vector.tensor_copy(ctx_sb[:], ctx_ps[0:HD, :])
                    ctxn = WD.tile([HD, SEGLEN], bf16, name=f"ctxn{j}_{s}",
                                   tag="ctxn", bufs=3)
                    nc.vector.tensor_tensor(ctxn[:], ctx_sb[:], bc_ps[:],
                                            ALU.mult)
                    # repack into the 3 K=128 out-proj tiles
                    for (po, c, toff, ln) in CTXP_PIECES[j]:
                        nc.sync.dma_start(ctxp[s][c][toff:toff + ln, :],
                                          ctxn[po:po + ln, :])

                def oproj(e, s):
                    o_ps = PSA.tile([128, SEGLEN], f32, name=f"ops{e}_{s}",
                                    tag="mm512")
                    for c in range(3):
                        nc.tensor.matmul(o_ps[:],
                                         wp_sb[:, EMBED * c + 128 * e:
                                               EMBED * c + 128 * (e + 1)],
                                         ctxp[s][c][:],
                                         start=(c == 0), stop=(c == 2))
                    oc = o_sb[s][:, SEGLEN * e:SEGLEN * (e + 1)]
                    if e % 2 == 0:
                        nc.vector.tensor_copy(oc, o_ps[:])
                    else:
                        nc.scalar.activation(oc, o_ps[:], AF.Identity)

                def ostore(s, half):
                    e0, e1 = (0, 5) if half == 0 else (5, KCH)
                    src = o_sb[s][:, SEGLEN * e0:SEGLEN * e1].rearrange(
                        "p (e t) -> p e t", t=SEGLEN)
                    dst = out_d[128 * e0:128 * e1,
                                SEGLEN * s:SEGLEN * (s + 1)].rearrange(
                        "(e p) t -> p e t", p=128)
                    nc.sync.dma_start(dst, src)

                # ---- phase B: packed qk projection + seg-0 v + rotary,
                # pipelined; seg-0 attention for head j gates only on packed
                # tiles <= j+1 so it overlaps the projection ----
                VCH = {0: [0, 1], 1: [2, 3], 2: [], 3: [], 4: []}
                for t in range(5):
                    for s in range(NSEG):
                        qkproj(t, s)
                    for m in VCH[t]:
                        vchunk(m)
                    ready = [t - 1] if t < 4 else [3, 4]
                    for tr in ready:
                        if tr < 0:
                            continue
                        for s in range(NSEG):
                            rotary(tr, s)
                    if t >= 2:
                        j = t - 2
                        scores(0, j)
                        ctx(0, j)

                # late phase: seg-1 v chunks + seg-1 attention + seg-0
                # out-proj all interleaved to keep the PE dense (HAM warm)
                scores(0, 3)
                ctx(0, 3)
                scores(1, 0)
                vchunk(4)
                vchunk(5)
                scores(1, 1)
                vchunk(6)
                vchunk(7)
                oproj(0, 0)
                oproj(1, 0)
                ctx(1, 0)
                oproj(2, 0)
                scores(1, 2)
                oproj(3, 0)
                oproj(4, 0)
                ctx(1, 1)
                ostore(0, 0)
                oproj(5, 0)
                oproj(6, 0)
                scores(1, 3)
                oproj(7, 0)
                oproj(8, 0)
                ctx(1, 2)
                oproj(9, 0)
                ostore(0, 1)
                ctx(1, 3)
                for e in range(KCH):
                    oproj(e, 1)
                    if e == 4:
                        ostore(1, 0)
                ostore(1, 1)

    nc.compile()
    return nc


def _prep_inputs(x, rotary_pos_emb, qkv_w, qkv_b):
    """Build per-core input shards (host-side layout/constant prep)."""
    x2 = np.asarray(x, np.float32).reshape(SEQ, EMBED)
    rope = np.asarray(rotary_pos_emb, np.float32)
    qkv_w = np.asarray(qkv_w, np.float32)
    qkv_b = np.asarray(qkv_b, np.float32)

    # packed rotary multipliers: packed row p = 80u + d -> r = d % 40
    r_idx = np.tile(np.arange(HD) % RH, NUNITS)      # [640]
    cos_full = np.cos(rope)[:, r_idx].T.astype(BF)   # [640, 2048]
    sin_full = np.sin(rope)[:, r_idx].T.astype(BF)

    # packed swap permutation (sign folded), block-diagonal per 80-row unit:
    # within a unit, row d<40 reads -(d+40), row d>=40 reads +(d-40)
    D = NUNITS * HD
    Pi = np.zeros((D, D), np.float32)
    for u in range(NUNITS):
        o = HD * u
        for i in range(RH):
            Pi[o + i, o + i + RH] = -1.0
            Pi[o + i + RH, o + i] = 1.0
    pit = np.ascontiguousarray(Pi.T).astype(BF)

    in_maps = []
    for c in range(N_CORES):
        sg, hg = divmod(c, HPC)
        toks = slice(TOK * sg, TOK * (sg + 1))
        heads = [HPC * hg + j for j in range(HPC)]

        xa = x2[toks].T                                   # [1280, 1024]

        # interleaved packing: unit 2j = q of head j, unit 2j+1 = k
        wqk = np.empty((EMBED, NUNITS * HD), np.float32)
        bias_flat = np.empty(NUNITS * HD, np.float32)
        for j, h in enumerate(heads):
            oq, ok = HD * 2 * j, HD * (2 * j + 1)
            wqk[:, oq:oq + HD] = qkv_w[HD * h:HD * (h + 1), :].T * SCALE
            bias_flat[oq:oq + HD] = qkv_b[HD * h:HD * (h + 1)] * SCALE
            ko = EMBED + HD * h
            wqk[:, ok:ok + HD] = qkv_w[ko:ko + HD, :].T
            bias_flat[ok:ok + HD] = qkv_b[ko:ko + HD]
        bias = np.ascontiguousarray(bias_flat.reshape(5, 128).T)

        # v weights: 97-wide blocks per head (80 v | 16 zero | 1 zero);
        # eviction adds vpat = (v-bias | 0 | 1.0) so the ctx matmul
        # accumulates the softmax denominator into partition 96
        wv = np.zeros((EMBED, VTOT), np.float32)
        vpat_row = np.zeros(VTOT, np.float32)
        for j, h in enumerate(heads):
            vo = 2 * EMBED + HD * h
            wv[:, VW * j:VW * j + HD] = qkv_w[vo:vo + HD, :].T
            vpat_row[VW * j:VW * j + HD] = qkv_b[vo:vo + HD]
            vpat_row[VW * j + 96] = 1.0
        vpat = np.ascontiguousarray(np.broadcast_to(vpat_row, (128, VTOT)))

        stream = np.concatenate([xa, wqk, wv], axis=1)    # [1280, SW]

        # wp packed for K=128 repacked ctx: rows = stacked head-dims
        # (320 real + 64 zero), 3 chunks of 128 side by side
        wp_cat = np.zeros((384, EMBED), np.float32)
        for j, h in enumerate(heads):
            wp_cat[HD * j:HD * (j + 1), :] = _PROJ_W[:, HD * h:HD * (h + 1)].T
        wp = np.zeros((128, 3 * EMBED), np.float32)
        for c_ in range(3):
            wp[:, EMBED * c_:EMBED * (c_ + 1)] = wp_cat[128 * c_:128 * (c_ + 1)]

        in_maps.append({
            "stream": np.ascontiguousarray(stream).astype(BF),
            "vpat": vpat.astype(BF),
            "wp": np.ascontiguousarray(wp).astype(BF),
            "biasqk": bias,
            "cosm": np.ascontiguousarray(cos_full[:, toks]),
            "sinm": np.ascontiguousarray(sin_full[:, toks]),
            "pit": pit,
        })
    return in_maps


_PROJ_W = None


def run_on_device(inputs, trace=False, trace_cores=None):
    """Shard, run on 8 NeuronCores, gather. Returns (output, BassKernelResults)."""
    global _PROJ_W
    from concourse import bass_utils

    x = np.asarray(inputs["x"], np.float32)
    cu = np.asarray(inputs["cu_seqlens"]).tolist()
    assert cu == [0, 512, 1024, 1536, 2048], (
        f"kernel compiled for 4x512 segments, got cu_seqlens={cu}")
    assert x.shape == (SEQ, 1, EMBED)

    _PROJ_W = np.asarray(inputs["proj_w"], np.float32)
    in_maps = _prep_inputs(x, inputs["rotary_pos_emb"],
                           inputs["qkv_w"], inputs["qkv_b"])

    if "nc" not in _CACHE:
        _CACHE["nc"] = _build_program()
    nc = _CACHE["nc"]

    kw = {}
    if trace:
        kw = dict(trace=True, trace_cores=trace_cores or [0])
    res = bass_utils.run_bass_kernel_spmd(nc, in_maps,
                                          core_ids=list(range(N_CORES)), **kw)

    proj_b = np.asarray(inputs["proj_b"], np.float32)
    out = np.empty((SEQ, EMBED), np.float32)
    for sg in range(2):
        acc = res.results[HPC * sg + 0]["outT"].astype(np.float32)
        for hg in range(1, HPC):
            acc = acc + res.results[HPC * sg + hg]["outT"].astype(np.float32)
        out[TOK * sg:TOK * (sg + 1)] = acc.T
    out += proj_b
    return out.reshape(SEQ, 1, EMBED), res


def kernel(**inputs):
    out, _ = run_on_device(inputs, trace=False)
    return out
